# revision 1
# baseline (speedup 1.0000x reference)
"""EvaAttention Trainium2 Bass kernel.

Strategy (per spec sharding hint): data-parallel over batch across 8 cores,
4 batches per core. All weights replicated; no collectives.

Per-core dataflow (all "transposed" to keep the contraction dim on SBUF
partitions without any on-chip input transposes):
  xT [768, 4*577]  (host-pretransposed x)
  A:  qkT[o, t] = wT[:, :1536].T @ xT       (q,k in head-transposed layout)
  B:  v[t, o]   = xT.T @ wT[:, 1536:]       (v in token-major layout)
  R:  partial RoPE on q,k via a constant permutation matmul (rot) + DVE
      mul/add.  Host pre-permutes each q/k head's d-dim to [evens, odds]
      (S is invariant under a shared per-head d-permutation), which turns
      timm's pairwise rot() into a 32-partition block swap.
  S:  S.T[k, q] per head on PE (contraction d=64), then E = exp(S.T/8) on ACT
      (no max subtraction: |S/8| <~ 6 for this data regime, exp is safe)
  PV: O.T[d, q] = [V | 1].T @ E accumulated over k-chunks; the appended ones
      column makes PE produce the softmax denominators as row 64 for free.
  renorm: O.T[0:64] * (1/denom) broadcast via a tiny SBUF->SBUF DMA
  P:  y[t, o] = oT.T @ pwT  (+proj_b), DMA out.
"""

import os
import numpy as np

import concourse.bacc as bacc
import concourse.tile as tile
import concourse.mybir as mybir
from concourse.bass_utils import run_bass_kernel_spmd

B, N, C = 32, 577, 768
H, D = 12, 64
NPT = 1
N_CORES = 8
NB = B // N_CORES  # batches per core
SCALE = D ** -0.5

MODE = os.environ.get("BASS_MM_MODE", "f32r")  # f32r | bf16 | f32
K_REP = int(os.environ.get("BASS_K_REP", "1"))

_f32 = mybir.dt.float32
if MODE == "bf16":
    _SB_DT = mybir.dt.bfloat16
elif MODE == "f32r":
    _SB_DT = mybir.dt.float32r  # matmul-feeding tiles natively fp32r
else:
    _SB_DT = mybir.dt.float32
_NP_SB = mybir.dt.np(_SB_DT)


def _mm(ap):
    return ap


def preprocess(x, rope, qkv_w, q_bias, v_bias, proj_w, proj_b):
    """Host-side prep: transposes, head-d permutation, rope tables."""
    perm = np.concatenate([np.arange(0, D, 2), np.arange(1, D, 2)])
    rows = np.arange(3 * C)
    for region in (0, 1):  # q, k head-d reorder; v untouched
        for h in range(H):
            base = region * C + h * D
            rows[base : base + D] = base + perm
    qkv_w_p = np.asarray(qkv_w, np.float32)[rows]
    qkvb_flat = np.concatenate(
        [np.asarray(q_bias, np.float32), np.zeros(C, np.float32),
         np.asarray(v_bias, np.float32)])[rows]
    # [128, 18] column-per-o-tile layout for per-partition ACT bias
    qkvb = np.ascontiguousarray(qkvb_flat.reshape(18, 128).T)

    wT = np.ascontiguousarray(qkv_w_p.T).astype(_NP_SB)          # [768, 2304]
    pwT = np.ascontiguousarray(np.asarray(proj_w, np.float32).T).astype(_NP_SB)

    rope = np.asarray(rope, np.float32)
    sinT = np.ascontiguousarray(rope[:, :D].T[perm])             # [64, 576]
    cosT = np.ascontiguousarray(rope[:, D:].T[perm])
    cosT2 = np.concatenate([cosT, cosT], 0).astype(_NP_SB)       # [128, 576]
    sinT2 = np.concatenate([sinT, sinT], 0).astype(_NP_SB)

    rotm = np.zeros((128, 128), np.float32)
    for blk in range(2):
        o = blk * 64
        for j in range(32):
            rotm[o + 32 + j, o + j] = -1.0   # out[j]    = -rhs[32+j]
            rotm[o + j, o + 32 + j] = 1.0    # out[32+j] = +rhs[j]
    rotm = rotm.astype(_NP_SB)

    x = np.asarray(x, np.float32)
    xTs = []
    for core in range(N_CORES):
        xc = x[core * NB : (core + 1) * NB]                      # [NB, 577, 768]
        xTs.append(np.ascontiguousarray(
            xc.transpose(2, 0, 1).reshape(C, NB * N)).astype(_NP_SB))

    vb = np.asarray(v_bias, np.float32)
    pb = np.asarray(proj_b, np.float32)
    return xTs, dict(wT=wT, pwT=pwT, qkvb=qkvb, vb=vb, pb=pb,
                     cosT2=cosT2, sinT2=sinT2, rotm=rotm)


def build(mode=MODE, k_rep=K_REP):
    nc = bacc.Bacc("TRN2", target_bir_lowering=False, debug=False,
                   num_devices=N_CORES)
    TT = NB * N  # tokens per core

    d_xT = nc.dram_tensor("xT", [C, TT], _SB_DT, kind="ExternalInput").ap()
    d_wT = nc.dram_tensor("wT", [C, 3 * C], _SB_DT, kind="ExternalInput").ap()
    d_pwT = nc.dram_tensor("pwT", [C, C], _SB_DT, kind="ExternalInput").ap()
    d_qkvb = nc.dram_tensor("qkvb", [128, 18], _f32, kind="ExternalInput").ap()
    d_vb = nc.dram_tensor("vb", [C], _f32, kind="ExternalInput").ap()
    d_pb = nc.dram_tensor("pb", [C], _f32, kind="ExternalInput").ap()
    d_cos = nc.dram_tensor("cosT2", [128, N - 1], _SB_DT, kind="ExternalInput").ap()
    d_sin = nc.dram_tensor("sinT2", [128, N - 1], _SB_DT, kind="ExternalInput").ap()
    d_rotm = nc.dram_tensor("rotm", [128, 128], _SB_DT, kind="ExternalInput").ap()
    d_out = nc.dram_tensor("out", [TT, C], _f32, kind="ExternalOutput").ap()

    Id = mybir.ActivationFunctionType.Identity
    Exp = mybir.ActivationFunctionType.Exp

    with tile.TileContext(nc) as tc:
        with tc.tile_pool(name="main", bufs=1) as pool, \
             tc.tile_pool(name="ps", bufs=1, space="PSUM") as pspool, \
             tc.tile_pool(name="dr", bufs=1, space="DRAM") as drpool:

            # ---- resident constants ----
            wT_sb = pool.tile([128, 6, 3 * C], _SB_DT, tag="wT")
            pwT_sb = pool.tile([128, 6, C], _SB_DT, tag="pwT")
            for c in range(6):
                nc.sync.dma_start(out=wT_sb[:, c, :], in_=d_wT[c * 128:(c + 1) * 128, :])
                nc.sync.dma_start(out=pwT_sb[:, c, :], in_=d_pwT[c * 128:(c + 1) * 128, :])
            qkvb_sb = pool.tile([128, 18], _f32, tag="qkvb")
            nc.sync.dma_start(out=qkvb_sb, in_=d_qkvb)
            import concourse.bass as bass_mod
            def _row_bc(ap, parts):
                return bass_mod.AP(tensor=ap.tensor, offset=ap.offset,
                                   ap=[[0, parts]] + list(ap.ap))
            vb_bc = pool.tile([128, C], _f32, tag="vb")
            nc.sync.dma_start(out=vb_bc, in_=_row_bc(d_vb, 128))
            pb_bc = pool.tile([128, C], _f32, tag="pb")
            nc.sync.dma_start(out=pb_bc, in_=_row_bc(d_pb, 128))
            cos_sb = pool.tile([128, N - 1], _SB_DT, tag="cos")
            nc.sync.dma_start(out=cos_sb, in_=d_cos)
            sin_sb = pool.tile([128, N - 1], _SB_DT, tag="sin")
            nc.sync.dma_start(out=sin_sb, in_=d_sin)
            rotm_sb = pool.tile([128, 128], _SB_DT, tag="rotm")
            nc.sync.dma_start(out=rotm_sb, in_=d_rotm)

            xres = mode == "bf16"
            if xres:
                xT_sb = pool.tile([128, 6, TT + 1], _SB_DT, tag="xT")
                nc.vector.memset(xT_sb[:, :, TT:TT + 1].bitcast(_f32), 0.0)
                for c in range(6):
                    nc.sync.dma_start(out=xT_sb[:, c, 0:TT],
                                      in_=d_xT[c * 128:(c + 1) * 128, :])

            dbuf = 2 if mode == "bf16" else 1
            sbuf2 = 2 if mode == "bf16" else 1

            def body():
                N2 = N + 1  # 578: fp32r matmuls need even free sizes
                for b in range(NB):
                    t0 = b * N
                    if xres:
                        xs = lambda c: xT_sb[:, c, t0:t0 + N2]
                    else:
                        xT_b = pool.tile([128, 6, N2], _SB_DT, tag="xb", bufs=sbuf2)
                        nc.vector.memset(xT_b[:, :, N:N2].bitcast(_f32), 0.0)
                        for c in range(6):
                            nc.sync.dma_start(
                                out=xT_b[:, c, 0:N],
                                in_=d_xT[c * 128:(c + 1) * 128, t0:t0 + N])
                        xs = lambda c: xT_b[:, c, :]

                    qk_sb = pool.tile([128, 12, N2], _SB_DT, tag="qk", bufs=dbuf)
                    v_sb = pool.tile([128, 5, H, D + 2], _SB_DT, tag="vsb", bufs=dbuf)
                    oT_sb = pool.tile([128, 6, N2], _SB_DT, tag="oT", bufs=dbuf)
                    nc.vector.memset(qk_sb[:, :, N:N2].bitcast(_f32), 0.0)
                    nc.vector.memset(oT_sb[:, :, N:N2].bitcast(_f32), 0.0)
                    # ones column everywhere, then zero the pad rows (65:128)
                    # of the last k-chunk, then restore row 64's ones (its V
                    # part is rewritten by phase B; the pad rows must be all
                    # zero so they drop out of the PV contraction).
                    nc.vector.memset(v_sb[:, :, :, D:D + 2].bitcast(_f32), 1.0)
                    nc.vector.memset(v_sb[64:128, 4, :, :].bitcast(_f32), 0.0)
                    nc.vector.memset(v_sb[64:65, 4, :, D:D + 2].bitcast(_f32), 1.0)

                    # ---- A: q,k (o-major) ----
                    for ot in range(12):
                        psA = pspool.tile([128, 512], _f32, tag="ps512", bufs=3)
                        psA2 = pspool.tile([128, 66], _f32, tag="ps65", bufs=3)
                        for c in range(6):
                            lhsT = wT_sb[:, c, ot * 128:(ot + 1) * 128]
                            nc.tensor.matmul(psA, lhsT, xs(c)[:, 0:512],
                                             start=c == 0, stop=c == 5)
                            nc.tensor.matmul(psA2, lhsT, xs(c)[:, 512:N2],
                                             start=c == 0, stop=c == 5)
                        bias = qkvb_sb[:, ot:ot + 1]
                        nc.scalar.activation(qk_sb[:, ot, 0:512], psA, Id, bias=bias)
                        nc.scalar.activation(qk_sb[:, ot, 512:N], psA2[:, 0:65],
                                             Id, bias=bias)

                    # ---- B: v (t-major) ----
                    for tt in range(5):
                        P = min(128, N - tt * 128)   # true rows (65 on last)
                        PM = P + (P % 2)             # padded matmul rows
                        psB = pspool.tile([128, 512], _f32, tag="ps512", bufs=3)
                        psB2 = pspool.tile([128, 256], _f32, tag="ps256", bufs=2)
                        for c in range(6):
                            lhsT = xs(c)[:, tt * 128:tt * 128 + PM]
                            nc.tensor.matmul(psB[:PM], lhsT,
                                             wT_sb[:, c, 1536:2048],
                                             start=c == 0, stop=c == 5)
                            nc.tensor.matmul(psB2[:PM], lhsT,
                                             wT_sb[:, c, 2048:2304],
                                             start=c == 0, stop=c == 5)
                        nc.vector.tensor_add(
                            v_sb[0:P, tt, 0:8, 0:D],
                            psB[:P].rearrange("p (h d) -> p h d", d=D),
                            vb_bc[0:P, 0:512].rearrange("p (h d) -> p h d", d=D))
                        nc.vector.tensor_add(
                            v_sb[0:P, tt, 8:12, 0:D],
                            psB2[:P].rearrange("p (h d) -> p h d", d=D),
                            vb_bc[0:P, 512:768].rearrange("p (h d) -> p h d", d=D))

                    # ---- R: rope on q,k ----
                    for ot in range(12):
                        psR = pspool.tile([128, 512], _f32, tag="ps512", bufs=3)
                        psR2 = pspool.tile([128, 66], _f32, tag="ps65", bufs=3)
                        nc.tensor.matmul(psR, rotm_sb, qk_sb[:, ot, 1:513],
                                         start=True, stop=True)
                        nc.tensor.matmul(psR2[:, 0:64], rotm_sb,
                                         qk_sb[:, ot, 513:N],
                                         start=True, stop=True)
                        tmp = pool.tile([128, N - 1], _SB_DT, tag="rtmp", bufs=sbuf2)
                        nc.vector.tensor_mul(tmp[:, 0:512], psR, sin_sb[:, 0:512])
                        nc.vector.tensor_mul(tmp[:, 512:N - 1], psR2[:, 0:64],
                                             sin_sb[:, 512:N - 1])
                        nc.vector.tensor_mul(qk_sb[:, ot, 1:N], qk_sb[:, ot, 1:N],
                                             cos_sb)
                        nc.vector.tensor_add(qk_sb[:, ot, 1:N], qk_sb[:, ot, 1:N],
                                             tmp)

                    # ---- attention per head ----
                    for h in range(H):
                        ot = h // 2
                        hb = (h % 2) * 64
                        E = pool.tile([128, 5, N2], _SB_DT, tag="E", bufs=2)
                        for kc in range(5):
                            KP = min(128, N + 1 - kc * 128)  # 128 or 66 padded
                            psS = pspool.tile([128, 512], _f32, tag="ps512", bufs=3)
                            psS2 = pspool.tile([128, 66], _f32, tag="ps65", bufs=3)
                            kk = qk_sb[hb:hb + 64, 6 + ot,
                                       kc * 128:kc * 128 + KP]
                            qq1 = qk_sb[hb:hb + 64, ot, 0:512]
                            qq2 = qk_sb[hb:hb + 64, ot, 512:N2]
                            nc.tensor.matmul(psS[:KP], kk, qq1, start=True, stop=True)
                            nc.tensor.matmul(psS2[:KP], kk, qq2, start=True, stop=True)
                            nc.scalar.activation(E[0:KP, kc, 0:512], psS[:KP],
                                                 Exp, scale=SCALE)
                            nc.scalar.activation(E[0:KP, kc, 512:N2], psS2[:KP],
                                                 Exp, scale=SCALE)
                        psO = pspool.tile([128, 512], _f32, tag="ps512", bufs=3)
                        psO2 = pspool.tile([128, 66], _f32, tag="ps65", bufs=3)
                        for kc in range(5):
                            KP = min(128, N + 1 - kc * 128)
                            vv = v_sb[0:KP, kc, h, :]
                            nc.tensor.matmul(psO[:D + 2], vv, E[0:KP, kc, 0:512],
                                             start=kc == 0, stop=kc == 4)
                            nc.tensor.matmul(psO2[:D + 2], vv, E[0:KP, kc, 512:N2],
                                             start=kc == 0, stop=kc == 4)
                        r = pool.tile([65, N], _f32, tag="r", bufs=sbuf2)
                        nc.vector.reciprocal(r[64:65, 0:512], psO[64:65, :])
                        nc.vector.reciprocal(r[64:65, 512:N], psO2[64:65, 0:65])
                        rrow = drpool.tile([1, N], _f32, tag="rrow", bufs=4)
                        nc.sync.dma_start(out=rrow, in_=r[64:65, :])
                        rbc = pool.tile([64, N], _f32, tag="rbc", bufs=sbuf2)
                        nc.sync.dma_start(out=rbc, in_=_row_bc(rrow[0, :], 64))
                        if hb == 0:
                            nc.vector.tensor_mul(oT_sb[0:64, ot, 0:512],
                                                 psO[0:64], rbc[:, 0:512])
                            nc.vector.tensor_mul(oT_sb[0:64, ot, 512:N],
                                                 psO2[0:64, 0:65], rbc[:, 512:N])
                        else:
                            otmp = pool.tile([64, N], _SB_DT, tag="otmp", bufs=sbuf2)
                            nc.vector.tensor_mul(otmp[:, 0:512], psO[0:64],
                                                 rbc[:, 0:512])
                            nc.vector.tensor_mul(otmp[:, 512:N], psO2[0:64, 0:65],
                                                 rbc[:, 512:N])
                            nc.sync.dma_start(out=oT_sb[64:128, ot, 0:N], in_=otmp)

                    # ---- P: output projection ----
                    for tt in range(5):
                        P = min(128, N - tt * 128)
                        PM = P + (P % 2)
                        psP = pspool.tile([128, 512], _f32, tag="ps512", bufs=3)
                        psP2 = pspool.tile([128, 256], _f32, tag="ps256", bufs=2)
                        for c in range(6):
                            lhsT = oT_sb[:, c, tt * 128:tt * 128 + PM]
                            nc.tensor.matmul(psP[:PM], lhsT, pwT_sb[:, c, 0:512],
                                             start=c == 0, stop=c == 5)
                            nc.tensor.matmul(psP2[:PM], lhsT, pwT_sb[:, c, 512:C],
                                             start=c == 0, stop=c == 5)
                        yt = pool.tile([128, C], _f32, tag="y", bufs=2)
                        nc.vector.tensor_add(yt[:P, 0:512], psP[:P], pb_bc[:P, 0:512])
                        nc.vector.tensor_add(yt[:P, 512:C], psP2[:P], pb_bc[:P, 512:C])
                        nc.sync.dma_start(
                            out=d_out[t0 + tt * 128:t0 + tt * 128 + P, :],
                            in_=yt[:P, :])

            if k_rep > 1:
                with tc.For_i(0, k_rep, 1):
                    body()
            else:
                body()

    nc.compile()
    return nc


_CACHE = {}


def _get_nc(mode=MODE, k_rep=K_REP):
    key = (mode, k_rep)
    if key not in _CACHE:
        _CACHE[key] = build(mode, k_rep)
    return _CACHE[key]


def kernel(**inputs) -> np.ndarray:
    xTs, pre = preprocess(**inputs)
    nc = _get_nc()
    shared = {k: pre[k] for k in
              ("wT", "pwT", "qkvb", "vb", "pb", "cosT2", "sinT2", "rotm")}
    in_maps = [dict(shared, xT=xTs[core]) for core in range(N_CORES)]
    res = run_bass_kernel_spmd(nc, in_maps, list(range(N_CORES)))
    out = np.concatenate(
        [res.results[c]["out"].reshape(NB, N, C) for c in range(N_CORES)], axis=0)
    return out



# revision 39
# speedup vs baseline: 2.3994x; 2.3994x over previous
"""EvaAttention Trainium2 Bass kernel (v2).

Strategy: data-parallel over batch across 8 cores, 4 batches per core.
All weights replicated; no collectives.  All matmul-feeding tiles bf16.

Per-core dataflow ("transposed" to keep contraction dims on SBUF partitions
with no on-chip input transposes):
  xT [768, 4*577]  (host-pretransposed x, resident in SBUF)
  A:  qkT[o, t] = wT[:, :1536].T @ xT       (q,k head-transposed)
  R:  partial RoPE via constant permutation matmul (rotm) + DVE/Pool mul/add.
      Host pre-permutes each q/k head's d-dim to [evens, odds], turning
      timm's pairwise rot() into a 32-partition block swap.
  B:  v[t, o]   = xT.T @ wT[:, 1536:]       (token-major)
  S:  S.T[k, q] per head on PE (contraction d=64); E = exp(S.T/8) on ACT
      (no max subtraction: |S/8| <~ 6 in this data regime, exp is safe)
  PV: O.T[d, q] = [V | 1].T @ E accumulated over k-chunks; the appended ones
      columns make PE produce softmax denominators as rows 64/65 for free.
  renorm: 1/denom via DVE reciprocal_approx_fast, DRAM-bounce broadcast DMA,
      then one Pool-engine mul straight into oT (even heads -> partitions
      0:64, odd heads -> 64:128; engines may shift between 32-aligned
      partition windows).
  P:  y[t, o] = oT.T @ pwT, DMA out.

Engine budget per batch: PE ~69us, ACT ~50us, DVE ~18us, Pool ~18us.
PSUM: 2 bufs x 2 banks general rotation + 2 bufs x 2 banks for PV = 8 banks.
Attention is emitted head-pipelined (S of head h interleaved with PV of head
h-1) so the PE tracks the ACT exp pace instead of serializing behind it.
"""

import os
import numpy as np

import concourse.bacc as bacc
import concourse.tile as tile
import concourse.mybir as mybir
import concourse.bass as bass_mod
from concourse.bass_utils import run_bass_kernel_spmd

B, N, C = 32, 577, 768
H, D = 12, 64
NPT = 1
N_CORES = 8
NB = B // N_CORES  # batches per core
TT = NB * N
SCALE = D ** -0.5

MODE = "bf16"
K_REP = int(os.environ.get("BASS_K_REP", "1"))

_f32 = mybir.dt.float32
_SB_DT = mybir.dt.bfloat16
_NP_SB = mybir.dt.np(_SB_DT)


def _row_bc(ap, parts):
    return bass_mod.AP(tensor=ap.tensor, offset=ap.offset,
                       ap=[[0, parts]] + list(ap.ap))


def preprocess(x, rope, qkv_w, q_bias, v_bias, proj_w, proj_b):
    """Host-side prep: transposes, head-d permutation, rope tables."""
    perm = np.concatenate([np.arange(0, D, 2), np.arange(1, D, 2)])
    rows = np.arange(3 * C)
    for region in (0, 1):  # q, k head-d reorder; v untouched
        for h in range(H):
            base = region * C + h * D
            rows[base : base + D] = base + perm
    qkv_w_p = np.asarray(qkv_w, np.float32)[rows]
    qkvb_flat = np.concatenate(
        [np.asarray(q_bias, np.float32), np.zeros(C, np.float32),
         np.asarray(v_bias, np.float32)])[rows]
    # [128, 18] column-per-o-tile layout for per-partition ACT bias
    qkvb = np.ascontiguousarray(qkvb_flat.reshape(18, 128).T)

    wT = np.ascontiguousarray(qkv_w_p.T).astype(_NP_SB)          # [768, 2304]
    pwT = np.ascontiguousarray(np.asarray(proj_w, np.float32).T).astype(_NP_SB)

    rope = np.asarray(rope, np.float32)
    sinT = np.ascontiguousarray(rope[:, :D].T[perm])             # [64, 576]
    cosT = np.ascontiguousarray(rope[:, D:].T[perm])
    cosT2 = np.concatenate([cosT, cosT], 0).astype(_NP_SB)       # [128, 576]
    sinT2 = np.concatenate([sinT, sinT], 0).astype(_NP_SB)

    rotm = np.zeros((128, 128), np.float32)
    for blk in range(2):
        o = blk * 64
        for j in range(32):
            rotm[o + 32 + j, o + j] = -1.0   # out[j]    = -rhs[32+j]
            rotm[o + j, o + 32 + j] = 1.0    # out[32+j] = +rhs[j]
    rotm = rotm.astype(_NP_SB)

    x = np.asarray(x, np.float32)
    xTs = []
    for core in range(N_CORES):
        xc = x[core * NB : (core + 1) * NB]                      # [NB, 577, 768]
        xTs.append(np.ascontiguousarray(
            xc.transpose(2, 0, 1).reshape(C, NB * N)).astype(_NP_SB))

    vb = np.asarray(v_bias, np.float32)
    pb = np.asarray(proj_b, np.float32)
    return xTs, dict(wT=wT, pwT=pwT, qkvb=qkvb, vb=vb, pb=pb,
                     cosT2=cosT2, sinT2=sinT2, rotm=rotm)


def build(has_vb=False, has_pb=False, k_rep=K_REP):
    nc = bacc.Bacc("TRN2", target_bir_lowering=False, debug=False,
                   num_devices=N_CORES)

    d_xT = nc.dram_tensor("xT", [C, TT], _SB_DT, kind="ExternalInput").ap()
    d_wT = nc.dram_tensor("wT", [C, 3 * C], _SB_DT, kind="ExternalInput").ap()
    d_pwT = nc.dram_tensor("pwT", [C, C], _SB_DT, kind="ExternalInput").ap()
    d_qkvb = nc.dram_tensor("qkvb", [128, 18], _f32, kind="ExternalInput").ap()
    d_vb = nc.dram_tensor("vb", [C], _f32, kind="ExternalInput").ap()
    d_pb = nc.dram_tensor("pb", [C], _f32, kind="ExternalInput").ap()
    d_cos = nc.dram_tensor("cosT2", [128, N - 1], _SB_DT, kind="ExternalInput").ap()
    d_sin = nc.dram_tensor("sinT2", [128, N - 1], _SB_DT, kind="ExternalInput").ap()
    d_rotm = nc.dram_tensor("rotm", [128, 128], _SB_DT, kind="ExternalInput").ap()
    d_out = nc.dram_tensor("out", [TT, C], _f32, kind="ExternalOutput").ap()

    Id = mybir.ActivationFunctionType.Identity
    Exp = mybir.ActivationFunctionType.Exp

    with tile.TileContext(nc) as tc:
        with tc.tile_pool(name="main", bufs=1) as pool, \
             tc.tile_pool(name="ps", bufs=1, space="PSUM") as pspool, \
             tc.tile_pool(name="dr", bufs=1, space="DRAM") as drpool:

            # ---- resident constants ----
            wT_sb = pool.tile([128, 6, 3 * C], _SB_DT, tag="wT")
            pwT_sb = pool.tile([128, 6, C], _SB_DT, tag="pwT")
            for c in range(6):
                nc.sync.dma_start(out=wT_sb[:, c, :], in_=d_wT[c * 128:(c + 1) * 128, :])
                nc.sync.dma_start(out=pwT_sb[:, c, :], in_=d_pwT[c * 128:(c + 1) * 128, :])
            qkvb_sb = pool.tile([128, 18], _f32, tag="qkvb")
            nc.sync.dma_start(out=qkvb_sb, in_=d_qkvb)
            cos_sb = pool.tile([128, N - 1], _SB_DT, tag="cos")
            nc.sync.dma_start(out=cos_sb, in_=d_cos)
            sin_sb = pool.tile([128, N - 1], _SB_DT, tag="sin")
            nc.sync.dma_start(out=sin_sb, in_=d_sin)
            rotm_sb = pool.tile([128, 128], _SB_DT, tag="rotm")
            nc.sync.dma_start(out=rotm_sb, in_=d_rotm)
            if has_vb:
                vb_bc = pool.tile([128, C], _f32, tag="vb")
                nc.sync.dma_start(out=vb_bc, in_=_row_bc(d_vb, 128))
            if has_pb:
                pb_bc = pool.tile([128, C], _f32, tag="pb")
                nc.sync.dma_start(out=pb_bc, in_=_row_bc(d_pb, 128))

            xT_sb = pool.tile([128, 6, TT], _SB_DT, tag="xT")
            for c in range(6):
                nc.sync.dma_start(out=xT_sb[:, c, :], in_=d_xT[c * 128:(c + 1) * 128, :])


            def body():
                qk_t, v_t, oT_t = {}, {}, {}

                def xs(b, c):
                    return xT_sb[:, c, b * N:b * N + N]

                def A_unit(b, ot, dve_evac=False):
                    qk_sb = qk_t[b]
                    psA = pspool.tile([128, 1024], _f32, tag="ps", bufs=2)
                    for c in range(6):
                        lhsT = wT_sb[:, c, ot * 128:(ot + 1) * 128]
                        nc.tensor.matmul(psA[:, 0:512], lhsT,
                                         xs(b, c)[:, 0:512],
                                         start=c == 0, stop=c == 5)
                        nc.tensor.matmul(psA[:, 512:N], lhsT,
                                         xs(b, c)[:, 512:N],
                                         start=c == 0, stop=c == 5)
                    if dve_evac:
                        # keep ACT free for the exp stream during attention
                        nc.vector.tensor_scalar_add(qk_sb[:, ot, :],
                                                    psA[:, 0:N],
                                                    qkvb_sb[:, ot:ot + 1])
                    else:
                        nc.scalar.activation(qk_sb[:, ot, :], psA[:, 0:N], Id,
                                             bias=qkvb_sb[:, ot:ot + 1])

                def A_phase(b):
                    qk_t[b] = pool.tile([128, 12, N], _SB_DT,
                                        tag="qk", bufs=2, name="qk_sb")
                    for ot in range(12):
                        A_unit(b, ot)

                def _R_unit(b, ot):
                    # psR lives in the psO tag (idle during RB) and is
                    # drained by ACT into bf16 so all three rope DVE ops run
                    # in the 2x all-bf16-SBUF mode
                    qk_sb = qk_t[b]
                    psR = pspool.tile([128, 1024], _f32, tag="psO", bufs=2,
                                      name="psR")
                    nc.tensor.matmul(psR[:, 0:512], rotm_sb, qk_sb[:, ot, 1:513],
                                     start=True, stop=True)
                    nc.tensor.matmul(psR[:, 512:576], rotm_sb, qk_sb[:, ot, 513:N],
                                     start=True, stop=True)
                    rotq = pool.tile([128, N - 1], _SB_DT, tag="rotq", bufs=2)
                    nc.scalar.copy(rotq, psR[:, 0:576])
                    tmp = pool.tile([128, N - 1], _SB_DT, tag="rtmp", bufs=2)
                    nc.vector.tensor_mul(tmp, rotq, sin_sb)
                    nc.vector.tensor_mul(qk_sb[:, ot, 1:N], qk_sb[:, ot, 1:N],
                                         cos_sb)
                    nc.vector.tensor_add(qk_sb[:, ot, 1:N], qk_sb[:, ot, 1:N],
                                         tmp)

                def _B_unit(b, tt):
                    v_sb = v_t[b]
                    P = min(128, N - tt * 128)   # 128 or 65 on the last
                    psB = pspool.tile([128, 1024], _f32, tag="ps", bufs=2)
                    for c in range(6):
                        lhsT = xs(b, c)[:, tt * 128:tt * 128 + P]
                        nc.tensor.matmul(psB[:P, 0:512], lhsT,
                                         wT_sb[:, c, 1536:2048],
                                         start=c == 0, stop=c == 5)
                        nc.tensor.matmul(psB[:P, 512:768], lhsT,
                                         wT_sb[:, c, 2048:2304],
                                         start=c == 0, stop=c == 5)
                    src = psB[:P, 0:768].rearrange("p (h d) -> p h d", d=D)
                    if has_vb:
                        nc.vector.tensor_add(
                            v_sb[0:P, tt, :, 0:D], src,
                            vb_bc[0:P, :].rearrange("p (h d) -> p h d", d=D))
                    else:
                        nc.scalar.copy(v_sb[0:P, tt, :, 0:D], src)

                def RB_phase(b):
                    # interleave R (DVE-paced) with B (pure PE+ACT) so the
                    # B matmuls fill the PE while R's vector ops drain; head
                    # 0's rope pair (ot 0, 6) lands first so attention can
                    # start immediately after
                    # ones columns feed the PV denominator rows; pad rows of
                    # the last k-chunk are never read (exact 65-row slices).
                    v_t[b] = v_sb = pool.tile([128, 5, H, D + 2], _SB_DT,
                                              tag="vsb", bufs=2, name="v_sb")
                    nc.vector.memset(v_sb[:, :, :, D:D + 2], 1.0)
                    for i in range(6):
                        _R_unit(b, i)
                        _R_unit(b, 6 + i)
                        if i < 5:
                            _B_unit(b, i)

                def attn_phase(b, nb=None, p_fill=None):
                    # head-pipelined S / PV / renorm; one A(nb) o-tile is
                    # woven into each head slot to keep the PE fed while the
                    # exp stream paces ACT
                    # renorm chain, staged over two head-slots so no engine
                    # head-of-line blocks:
                    #   stage1(h): DVE copies denom row out of PSUM,
                    #              row -> DRAM -> [128,5] SBUF
                    #   stage2(h): cheap [128,5] reciprocal, back through
                    #              DRAM, broadcast to 64 partitions, DVE mul
                    qk_sb, v_sb = qk_t[b], v_t[b]
                    oT_t[b] = oT_sb = pool.tile([128, 6, N], _SB_DT, tag="oT",
                                                bufs=2, name="oT_sb")
                    if nb is not None:
                        qk_t[nb] = pool.tile([128, 12, N], _SB_DT,
                                             tag="qk", bufs=2, name="qk_sb")
                    E_t = [None, None]
                    psO_t = [None, None]
                    rsb_t = [None, None]
                    oun_t = [None, None]
                    for h in range(14):
                        if h < 12:
                            E_t[h % 2] = pool.tile([128, 5, N], _SB_DT,
                                                   tag="E", bufs=2, name="E")
                        if 1 <= h <= 12:
                            psO_t[(h - 1) % 2] = pspool.tile(
                                [128, 1024], _f32, tag="psO", bufs=2, name="psO")
                        for kc in range(5):
                            KP = min(128, N - kc * 128)
                            if h < 12:
                                ot = h // 2
                                hb = (h % 2) * 64
                                psS = pspool.tile([128, 1024], _f32,
                                                  tag="ps", bufs=2)
                                kk = qk_sb[hb:hb + 64, 6 + ot,
                                           kc * 128:kc * 128 + KP]
                                nc.tensor.matmul(psS[:KP, 0:512], kk,
                                                 qk_sb[hb:hb + 64, ot, 0:512],
                                                 start=True, stop=True)
                                nc.tensor.matmul(psS[:KP, 512:N], kk,
                                                 qk_sb[hb:hb + 64, ot, 512:N],
                                                 start=True, stop=True)
                                nc.scalar.activation(E_t[h % 2][0:KP, kc, :],
                                                     psS[:KP, 0:N], Exp,
                                                     scale=SCALE)
                            if 1 <= h <= 12:
                                hp = h - 1
                                E_p, psO = E_t[hp % 2], psO_t[hp % 2]
                                vv = v_sb[0:KP, kc, hp, :]
                                nc.tensor.matmul(psO[:D + 2, 0:512], vv,
                                                 E_p[0:KP, kc, 0:512],
                                                 start=kc == 0, stop=kc == 4)
                                nc.tensor.matmul(psO[:D + 2, 512:N], vv,
                                                 E_p[0:KP, kc, 512:N],
                                                 start=kc == 0, stop=kc == 4)
                        if nb is not None and h < 12:
                            A_unit(nb, h, dve_evac=True)
                        elif p_fill is not None and 3 <= h <= 11 and h % 2 == 1:
                            P_unit(p_fill, (h - 3) // 2, dve_evac=True)
                        if 1 <= h <= 12:
                            hp = h - 1  # renorm stage 1
                            psO = psO_t[hp % 2]
                            dn = pool.tile([128, 640], _f32, tag="dn", bufs=2)
                            nc.vector.memset(dn[64:65, N:640], 1.0)
                            nc.vector.tensor_copy(dn[64:65, 0:N], psO[64:65, 0:N])
                            oun = pool.tile([128, N], _SB_DT, tag="oun",
                                            bufs=2, name="oun")
                            oun_t[hp % 2] = oun
                            nc.vector.tensor_copy(oun[0:64, :], psO[0:64, 0:N])
                            rrow = drpool.tile([1, 640], _f32, tag="rrow",
                                               bufs=2)
                            nc.sync.dma_start(out=rrow, in_=dn[64:65, :])
                            rsb = pool.tile([128, 5], _f32, tag="rsb", bufs=2,
                                            name="rsb")
                            rsb_t[hp % 2] = rsb
                            nc.sync.dma_start(out=rsb, in_=rrow.rearrange(
                                "o (p f) -> (o p) f", f=5))
                        if 2 <= h <= 13:
                            hq = h - 2  # renorm stage 2
                            ot = hq // 2
                            hb = (hq % 2) * 64
                            rinv = pool.tile([128, 5], _f32, tag="rinv", bufs=2)
                            nc.vector.reciprocal(rinv, rsb_t[hq % 2])
                            rrow2 = drpool.tile([1, 640], _f32, tag="rrow2",
                                                bufs=2)
                            nc.sync.dma_start(out=rrow2.rearrange(
                                "o (p f) -> (o p) f", f=5), in_=rinv)
                            rbc = pool.tile([128, N], _f32, tag="rbc", bufs=2)
                            nc.sync.dma_start(out=rbc[0:64, :],
                                              in_=_row_bc(rrow2[0, 0:N], 64))
                            nc.gpsimd.tensor_mul(oT_sb[hb:hb + 64, ot, :],
                                                 oun_t[hq % 2][0:64, :],
                                                 rbc[0:64, :])

                def P_unit(b, tt, dve_evac=False):
                    t0 = b * N
                    oT_sb = oT_t[b]
                    P = min(128, N - tt * 128)
                    psP = pspool.tile([128, 1024], _f32, tag="ps", bufs=2)
                    for c in range(6):
                        lhsT = oT_sb[:, c, tt * 128:tt * 128 + P]
                        nc.tensor.matmul(psP[:P, 0:512], lhsT,
                                         pwT_sb[:, c, 0:512],
                                         start=c == 0, stop=c == 5)
                        nc.tensor.matmul(psP[:P, 512:768], lhsT,
                                         pwT_sb[:, c, 512:C],
                                         start=c == 0, stop=c == 5)
                    yt = pool.tile([128, C], _f32, tag="y", bufs=2)
                    if has_pb:
                        nc.vector.tensor_add(yt[:P, :], psP[:P, 0:768],
                                             pb_bc[:P, :])
                    elif dve_evac:
                        nc.vector.tensor_copy(yt[:P, :], psP[:P, 0:768])
                    else:
                        nc.scalar.copy(yt[:P, :], psP[:P, 0:768])
                    nc.sync.dma_start(
                        out=d_out[t0 + tt * 128:t0 + tt * 128 + P, :],
                        in_=yt[:P, :])

                def P_phase(b):
                    for tt in range(5):
                        P_unit(b, tt)

                # software pipeline: A(b+1) is woven into attention(b)'s head
                # slots (the exp stream paces ACT there, leaving PE slack);
                # RB(b+1) lands between attention(b) and P(b) so the last
                # heads' renorm chains drain under RB's matmuls.
                A_phase(0)
                RB_phase(0)
                for b in range(NB):
                    nb = b + 1 if b + 1 < NB else None
                    attn_phase(b, nb, p_fill=b - 1 if nb is None else None)
                    if nb is not None:
                        RB_phase(nb)
                    if b == NB - 2:
                        continue  # P(NB-2) is woven into attn(NB-1) instead
                    P_phase(b)

            if k_rep > 1:
                with tc.For_i(0, k_rep, 1):
                    body()
            else:
                body()

    nc.compile()
    return nc


_CACHE = {}


def _get_nc(mode=MODE, k_rep=K_REP, has_vb=False, has_pb=False):
    key = (mode, k_rep, has_vb, has_pb)
    if key not in _CACHE:
        _CACHE[key] = build(has_vb, has_pb, k_rep)
    return _CACHE[key]


def kernel(**inputs) -> np.ndarray:
    xTs, pre = preprocess(**inputs)
    has_vb = bool(np.any(pre["vb"]))
    has_pb = bool(np.any(pre["pb"]))
    nc = _get_nc(MODE, K_REP, has_vb, has_pb)
    shared = {k: pre[k] for k in
              ("wT", "pwT", "qkvb", "vb", "pb", "cosT2", "sinT2", "rotm")}
    in_maps = [dict(shared, xT=xTs[core]) for core in range(N_CORES)]
    res = run_bass_kernel_spmd(nc, in_maps, list(range(N_CORES)))
    out = np.concatenate(
        [res.results[c]["out"].reshape(NB, N, C) for c in range(N_CORES)], axis=0)
    return out
